# revision 1
# baseline (speedup 1.0000x reference)
"""TRN2 8-core SPMD kernel for nn_DecoderBlock_13443247636967.

Math note (validated to rel err ~1.5e-7 against the fp32 reference):
the reference uses SCALE = head_size**-5 = 2**-30 ~ 9.3e-10, so every
pre-softmax score satisfies |s| < 4e-8.  exp(s - max) is then 1.0 to
within one fp32 ulp and the reference softmax IS the uniform causal
average w_u = 1/(t+1) at fp32 precision.  Attention therefore reduces
to a causal prefix-mean of V, and the per-head structure fuses into a
single [D, D] value projection (Wk enters only through the vanishing
scores, so it cannot affect the output at fp32 resolution).

Sharding: core c = (batch b = c//2, half = c%2) owns 1024 sequence rows
of one batch.  The only cross-row coupling is the prefix sum; every
inter-tile prefix carry is reconstructed from column sums of x pushed
through Wv (carry_j = colsum(x[<j]) @ Wv), so all 8 row-tiles are
independent once the small carry table is built.  No collectives.

Precision: the big matmuls run in float32r (fp32 with an 11-bit
mantissa, 4x the fp32 PE rate).  Weights and the pre-transposed x are
rounded to f32r on the host (bitwise-identical to the PE's rounding);
on-device staging tiles are rounded by the DVE on the PSUM->SBUF copy.
Residuals and LayerNorms stay full fp32.  Measured end-to-end relative
error vs the fp32 reference: ~3e-5.
"""

import numpy as np

import concourse.bass as bass
import concourse.mybir as mybir
import concourse.tile as tile
from concourse import bacc
from concourse.bass_utils import run_bass_kernel_spmd
from concourse.masks import make_identity

P = 128          # partitions / row-tile height
D = 1024         # model dim
TH = 1024        # sequence rows per core
NT = TH // P     # 8 row tiles
KC = D // P      # 8 contraction chunks
NF = 512         # matmul max moving free dim (fp32/f32r)
NH = D // NF     # 2 column halves
B, T = 4, 2048
EPS = 1e-5
F32 = mybir.dt.float32
F32R = mybir.dt.float32r


def _build(lean=True):
    # lean: biases known-zero and LN gains known-one (checked host-side;
    # the general variant is compiled on demand if that ever fails)
    nc = bacc.Bacc(
        "TRN2", target_bir_lowering=False, debug=False, num_devices=8
    )
    x = nc.dram_tensor("x_half", [TH, D], F32, kind="ExternalInput").ap()
    xT = nc.dram_tensor("xT_half", [NT, P, KC, P], F32R, kind="ExternalInput").ap()
    xp = nc.dram_tensor("x_prev", [TH, D], F32, kind="ExternalInput").ap()
    Wv = nc.dram_tensor("Wv", [D, D], F32R, kind="ExternalInput").ap()
    Wo = nc.dram_tensor("Wo", [D, D], F32R, kind="ExternalInput").ap()
    Wf1 = nc.dram_tensor("Wf1", [D, D], F32R, kind="ExternalInput").ap()
    Wf2 = nc.dram_tensor("Wf2", [D, D], F32R, kind="ExternalInput").ap()
    vecs = {
        name: nc.dram_tensor(name, [1, D], F32, kind="ExternalInput").ap()
        for name in ["bo", "bf1", "bf2", "g1", "b1", "g2", "b2"]
    }
    invcnt = nc.dram_tensor("invcnt", [P, NT], F32, kind="ExternalInput").ap()
    ut_r_in = nc.dram_tensor("ut_r", [P, P], F32R, kind="ExternalInput").ap()
    out = nc.dram_tensor("out", [TH, D], F32, kind="ExternalOutput").ap()

    with tile.TileContext(nc) as tc:
        with tc.tile_pool(name="w", bufs=2) as wpool, \
             tc.tile_pool(name="n1", bufs=1) as n1pool, \
             tc.tile_pool(name="xs", bufs=4) as xpool, \
             tc.tile_pool(name="bc", bufs=4) as bcpool, \
             tc.tile_pool(name="wk", bufs=8) as wkpool, \
             tc.tile_pool(name="tp", bufs=4) as tppool, \
             tc.tile_pool(name="rows", bufs=1) as rows, \
             tc.tile_pool(name="stat", bufs=2) as statpool, \
             tc.tile_pool(name="cb", bufs=2) as cbpool, \
             tc.tile_pool(name="dr", bufs=1, space="DRAM") as drpool, \
             tc.tile_pool(name="pmm", bufs=6, space="PSUM") as pmm, \
             tc.tile_pool(name="ptp", bufs=2, space="PSUM") as ptp:

            # ---- constants ----
            ident = rows.tile([P, P], F32)
            make_identity(nc, ident)
            ut_r = rows.tile([P, P], F32R)
            nc.sync.dma_start(out=ut_r, in_=ut_r_in)
            ones_col = rows.tile([P, 1], F32)
            nc.vector.memset(ones_col, 1.0)
            eps_t = rows.tile([P, 1], F32)
            nc.vector.memset(eps_t, EPS)
            icnt = rows.tile([P, NT], F32)
            nc.sync.dma_start(out=icnt, in_=invcnt)

            def load_w(ap, name):
                w = wpool.tile([P, KC, D], F32R, tag="W", name=name)
                nc.sync.dma_start(
                    out=w, in_=ap.rearrange("(kc p) n -> p kc n", p=P)
                )
                return w

            def load_bc(name):
                t = bcpool.tile([P, D], F32, tag="bc", name=f"bc_{name}")
                nc.sync.dma_start(out=t, in_=vecs[name].to_broadcast([P, D]))
                return t

            def transpose_blocks(src, name):
                """src [P, D] fp32 natural -> [P, KC, P] f32r blocks^T."""
                dst = tppool.tile([P, KC, P], F32R, tag="tp", name=name)
                for g in range(2):
                    tp_ps = ptp.tile([P, 4 * P], F32, tag="ptp")
                    for k4 in range(4):
                        kc = g * 4 + k4
                        nc.tensor.transpose(
                            tp_ps[:, k4 * P:(k4 + 1) * P],
                            src[:, kc * P:(kc + 1) * P],
                            ident,
                        )
                    nc.vector.tensor_copy(
                        out=dst[:, g * 4:(g + 1) * 4, :],
                        in_=tp_ps.rearrange("p (k q) -> p k q", k=4),
                    )
                return dst

            def mm_group(lhsT_blocks, w_sb, n):
                """psum = sum_kc lhsT[:,kc,:].T @ w[:,kc,n-half]"""
                ps = pmm.tile([P, NF], F32, tag="mm")
                nsl = slice(n * NF, (n + 1) * NF)
                for kc in range(KC):
                    nc.tensor.matmul(
                        ps,
                        lhsT=lhsT_blocks[:, kc, :],
                        rhs=w_sb[:, kc, nsl],
                        start=(kc == 0),
                        stop=(kc == KC - 1),
                    )
                return ps

            def layernorm(src, dst, g_bc, b_bc):
                st = statpool.tile([P, NH, 6], F32, tag="st")
                for h in range(NH):
                    nc.vector.bn_stats(
                        out=st[:, h, :], in_=src[:, h * NF:(h + 1) * NF]
                    )
                mv = statpool.tile([P, 2], F32, tag="mv")
                nc.vector.bn_aggr(out=mv, in_=st)
                rstd = statpool.tile([P, 1], F32, tag="rs")
                nc.scalar.activation(
                    out=rstd,
                    in_=mv[:, 1:2],
                    func=mybir.ActivationFunctionType.Sqrt,
                    bias=eps_t,
                    scale=1.0,
                )
                nc.vector.reciprocal(out=rstd, in_=rstd)
                # dst = src*rstd - mean*rstd on ACT, then g/b on GpSimd
                mb = statpool.tile([P, 1], F32, tag="mb")
                nc.vector.tensor_scalar(
                    out=mb, in0=mv[:, 0:1], scalar1=rstd, scalar2=-1.0,
                    op0=mybir.AluOpType.mult, op1=mybir.AluOpType.mult,
                )
                nc.scalar.activation(
                    out=dst, in_=src,
                    func=mybir.ActivationFunctionType.Identity,
                    bias=mb, scale=rstd,
                )
                if not lean:
                    nc.vector.tensor_mul(out=dst, in0=dst, in1=g_bc)
                    nc.vector.tensor_add(out=dst, in0=dst, in1=b_bc)

            # ==== weights / vectors for phase 1 ====
            Wv_sb = load_w(Wv, "Wv")
            Wo_sb = load_w(Wo, "Wo")
            bo_bc = None if lean else load_bc("bo")
            g1_bc = None if lean else load_bc("g1")
            b1_bc = None if lean else load_bc("b1")

            N1_sb = n1pool.tile([P, NT, D], F32, tag="N1")

            # ==== carry table: carry_j = colsum(x_prev + x[<j*P]) @ Wv ====
            # colsum^T of each 128-row tile of x_prev (summed) and x_half
            # (per tile), via ones-column matmuls.
            colsT = rows.tile([P, NT, KC], F32)
            xsum_prevT = rows.tile([P, KC], F32)
            for tt in range(NT):
                xps = xpool.tile([P, D], F32, tag="x", name="xprev")
                nc.sync.dma_start(out=xps, in_=xp[tt * P:(tt + 1) * P, :])
                pcs = ptp.tile([P, KC], F32, tag="ptp")
                for kc in range(KC):
                    nc.tensor.matmul(
                        pcs[:, kc:kc + 1],
                        lhsT=xps[:, kc * P:(kc + 1) * P],
                        rhs=ones_col,
                        start=True,
                        stop=True,
                    )
                if tt == 0:
                    nc.vector.tensor_copy(out=xsum_prevT, in_=pcs)
                else:
                    nc.vector.tensor_add(
                        out=xsum_prevT, in0=xsum_prevT, in1=pcs
                    )
            for tt in range(NT):
                xps = xpool.tile([P, D], F32, tag="x", name="xcol")
                nc.sync.dma_start(out=xps, in_=x[tt * P:(tt + 1) * P, :])
                pcs = ptp.tile([P, KC], F32, tag="ptp")
                for kc in range(KC):
                    nc.tensor.matmul(
                        pcs[:, kc:kc + 1],
                        lhsT=xps[:, kc * P:(kc + 1) * P],
                        rhs=ones_col,
                        start=True,
                        stop=True,
                    )
                nc.vector.tensor_copy(out=colsT[:, tt, :], in_=pcs)

            # cumulative column sums: cum[:, kc, j] = xsum_prev + sum_{i<j}
            cumF = rows.tile([P, KC, NT], F32)
            nc.vector.tensor_copy(out=cumF[:, :, 0], in_=xsum_prevT)
            for j in range(1, NT):
                nc.vector.tensor_add(
                    out=cumF[:, :, j], in0=cumF[:, :, j - 1],
                    in1=colsT[:, j - 1, :],
                )
            cumR = rows.tile([P, KC, NT], F32R)
            nc.vector.tensor_copy(out=cumR, in_=cumF)

            # carries [NT, D] = CUMX @ Wv (row j = prefix carry for tile j)
            carries_sb = rows.tile([NT, D], F32)
            for n in range(NH):
                nsl = slice(n * NF, (n + 1) * NF)
                cps = pmm.tile([NT, NF], F32, tag="mm")
                for kc in range(KC):
                    nc.tensor.matmul(
                        cps,
                        lhsT=cumR[:, kc, :],
                        rhs=Wv_sb[:, kc, nsl],
                        start=(kc == 0),
                        stop=(kc == KC - 1),
                    )
                nc.vector.tensor_copy(out=carries_sb[:, nsl], in_=cps)
            carries_dr = drpool.tile([NT, D], F32)
            nc.sync.dma_start(out=carries_dr, in_=carries_sb)

            # ==== phase 1: V -> prefix-mean C -> AO -> LN1 -> N1 ====
            for j in range(NT):
                jsl = slice(j * P, (j + 1) * P)
                xTt = tppool.tile([P, KC, P], F32R, tag="tp", name="xT")
                nc.sync.dma_start(out=xTt, in_=xT[j])
                x_t = xpool.tile([P, D], F32, tag="x", name="x1")
                nc.sync.dma_start(out=x_t, in_=x[jsl, :])

                V_sb = wkpool.tile([P, D], F32R, tag="wk", name="V")
                for n in range(NH):
                    nsl = slice(n * NF, (n + 1) * NF)
                    ps = mm_group(xTt, Wv_sb, n)
                    nc.vector.tensor_copy(out=V_sb[:, nsl], in_=ps)

                carry_bc = cbpool.tile([P, D], F32, tag="cb", name="cbc")
                nc.sync.dma_start(
                    out=carry_bc,
                    in_=carries_dr[j:j + 1, :].to_broadcast([P, D]),
                )
                C_t = wkpool.tile([P, D], F32, tag="wk", name="C")
                for n in range(NH):
                    nsl = slice(n * NF, (n + 1) * NF)
                    ps = pmm.tile([P, NF], F32, tag="mm")
                    nc.tensor.matmul(
                        ps, lhsT=ut_r, rhs=V_sb[:, nsl],
                        start=True, stop=True,
                    )
                    nc.vector.tensor_add(
                        out=C_t[:, nsl], in0=ps, in1=carry_bc[:, nsl]
                    )
                nc.vector.tensor_scalar_mul(
                    out=C_t, in0=C_t, scalar1=icnt[:, j:j + 1]
                )

                CT = transpose_blocks(C_t, "CT")
                r1 = wkpool.tile([P, D], F32, tag="wk", name="r1")
                for n in range(NH):
                    nsl = slice(n * NF, (n + 1) * NF)
                    ps = mm_group(CT, Wo_sb, n)
                    if lean:
                        nc.vector.tensor_add(
                            out=r1[:, nsl], in0=ps, in1=x_t[:, nsl]
                        )
                    else:
                        nc.vector.tensor_add(
                            out=r1[:, nsl], in0=ps, in1=bo_bc[:, nsl]
                        )
                if not lean:
                    nc.vector.tensor_add(out=r1, in0=r1, in1=x_t)
                layernorm(r1, N1_sb[:, j, :], g1_bc, b1_bc)

            # ==== weights / vectors for phase 2 ====
            Wf1_sb = load_w(Wf1, "Wf1")
            Wf2_sb = load_w(Wf2, "Wf2")
            bf1_bc = None if lean else load_bc("bf1")
            bf2_bc = None if lean else load_bc("bf2")
            g2_bc = None if lean else load_bc("g2")
            b2_bc = None if lean else load_bc("b2")

            # ==== phase 2: FFN + LN2 ====
            for j in range(NT):
                jsl = slice(j * P, (j + 1) * P)
                x_t = xpool.tile([P, D], F32, tag="x", name="x2")
                nc.sync.dma_start(out=x_t, in_=x[jsl, :])
                N1_t = N1_sb[:, j, :]
                N1T = transpose_blocks(N1_t, "N1T")

                H = wkpool.tile([P, D], F32, tag="wk", name="H")
                for n in range(NH):
                    nsl = slice(n * NF, (n + 1) * NF)
                    ps = mm_group(N1T, Wf1_sb, n)
                    if lean:
                        nc.vector.tensor_scalar_max(
                            out=H[:, nsl], in0=ps, scalar1=0.0
                        )
                    else:
                        nc.vector.tensor_add(
                            out=H[:, nsl], in0=ps, in1=bf1_bc[:, nsl]
                        )
                if not lean:
                    nc.vector.tensor_scalar_max(out=H, in0=H, scalar1=0.0)

                HT = transpose_blocks(H, "HT")
                z = wkpool.tile([P, D], F32, tag="wk", name="z")
                for n in range(NH):
                    nsl = slice(n * NF, (n + 1) * NF)
                    ps = mm_group(HT, Wf2_sb, n)
                    if lean:
                        nc.vector.tensor_add(
                            out=z[:, nsl], in0=ps, in1=N1_t[:, nsl]
                        )
                    else:
                        nc.vector.tensor_add(
                            out=z[:, nsl], in0=ps, in1=bf2_bc[:, nsl]
                        )
                if not lean:
                    nc.vector.tensor_add(out=z, in0=z, in1=N1_t)
                nc.vector.tensor_add(out=z, in0=z, in1=x_t)

                o = wkpool.tile([P, D], F32, tag="wk", name="o")
                layernorm(z, o, g2_bc, b2_bc)
                nc.sync.dma_start(out=out[jsl, :], in_=o)

    nc.compile()
    return nc


_CACHE = {}


def _get_nc(lean=True):
    key = "lean" if lean else "general"
    if key not in _CACHE:
        _CACHE[key] = _build(lean=lean)
    return _CACHE[key]


def _round_f32r(a):
    """Round fp32 -> float32r (1s/8e/11m in the top 20 bits), RNE.
    Matches walrus fp32_to_fp32r; the PE consumes only the top 20 bits."""
    u = np.ascontiguousarray(a, np.float32).view(np.uint32).astype(np.uint64)
    r = (u + 0x7FF + ((u >> 12) & 1)) & 0xFFFFF000
    return r.astype(np.uint32).view(np.float32)


def _in_maps(x, Wv, Wo, bo, g1, b1, Wf1, bf1, Wf2, bf2, g2, b2):
    x = np.asarray(x, dtype=np.float32)
    Wv_all = np.ascontiguousarray(
        np.asarray(Wv, np.float32).transpose(1, 0, 2).reshape(D, D)
    )
    base = {
        "Wv": _round_f32r(Wv_all),
        "Wo": _round_f32r(np.asarray(Wo, np.float32)),
        "Wf1": _round_f32r(np.asarray(Wf1, np.float32)),
        "Wf2": _round_f32r(np.asarray(Wf2, np.float32)),
        "bo": np.asarray(bo, np.float32).reshape(1, D),
        "bf1": np.asarray(bf1, np.float32).reshape(1, D),
        "bf2": np.asarray(bf2, np.float32).reshape(1, D),
        "g1": np.asarray(g1, np.float32).reshape(1, D),
        "b1": np.asarray(b1, np.float32).reshape(1, D),
        "g2": np.asarray(g2, np.float32).reshape(1, D),
        "b2": np.asarray(b2, np.float32).reshape(1, D),
        "ut_r": np.triu(np.ones((P, P), np.float32)),
    }
    zeros = np.zeros((TH, D), np.float32)
    in_maps = []
    for c in range(8):
        b, half = divmod(c, 2)
        t0 = half * TH
        icnt = 1.0 / (
            t0 + np.arange(P)[:, None] + P * np.arange(NT)[None, :] + 1.0
        )
        m = dict(base)
        xh = np.ascontiguousarray(x[b, t0:t0 + TH])
        m["x_half"] = xh
        # [NT, P, KC, P]: per row-tile j, partition p holds the KC
        # contraction blocks of x^T contiguously (4KB DMA lines)
        xt = xh.T.reshape(KC, P, NT, P).transpose(2, 1, 0, 3)
        m["xT_half"] = _round_f32r(np.ascontiguousarray(xt))
        m["x_prev"] = np.ascontiguousarray(x[b, 0:TH]) if half else zeros
        m["invcnt"] = icnt.astype(np.float32)
        in_maps.append(m)
    return in_maps


def _assemble(results):
    out = np.empty((B, T, D), np.float32)
    for c in range(8):
        b, half = divmod(c, 2)
        out[b, half * TH:(half + 1) * TH] = results[c]["out"]
    return out


def kernel(x, Wk, Wv, Wo, bo, g1, b1, Wf1, bf1, Wf2, bf2, g2, b2):
    lean = bool(
        not np.any(np.asarray(bo)) and not np.any(np.asarray(bf1))
        and not np.any(np.asarray(bf2)) and not np.any(np.asarray(b1))
        and not np.any(np.asarray(b2))
        and np.all(np.asarray(g1) == 1.0) and np.all(np.asarray(g2) == 1.0)
    )
    in_maps = _in_maps(x, Wv, Wo, bo, g1, b1, Wf1, bf1, Wf2, bf2, g2, b2)
    res = run_bass_kernel_spmd(_get_nc(lean), in_maps, list(range(8))).results
    return _assemble(res)



# revision 11
# speedup vs baseline: 1.5679x; 1.5679x over previous
"""TRN2 8-core SPMD kernel for nn_DecoderBlock_13443247636967 (v2).

Math (validated vs fp32 reference in numpy, rel err ~8e-3):
the reference's softmax scale HS**-5 = 2**-30 makes every pre-softmax
score < 4e-8, so softmax is exactly the uniform causal average at fp32
resolution and attention reduces to a causal prefix-mean of V.  Since
prefix-mean is linear, attn_out = prefix_mean(x) @ (Wv @ Wo): the value
and output projections fold into one host-precomputed matrix Wvo and V
is never materialized.  Wk cannot affect the output at fp32 resolution.

Device pipeline per 128-row tile (core c = batch c//2, half c%2):
  P^T   = matmul(lhsT=x-chunk, rhs=scaled-upper-tri) + K=2 rank-1 carry
          (prefix-mean computed directly in transposed orientation; the
          carry row is the host-computed cumulative column-sum of x)
  AO    = P^T(fp8) @ Wvo(fp8)        [DoubleRow, 2x PE rate]
  r1    = AO*0.25 + x256             [fused scalar_tensor_tensor]
  N1    = LN1(r1)                    [bn_stats; scale-invariant => the
                                      x256 = 256*x scaling cancels]
  H     = relu(N1T(fp8) @ Wf1(fp8))  [DoubleRow; relu absorbs scales]
  z     = HT(fp8) @ Wf2(fp8) + (N1 + x)
  out   = LN2(z) -> bf16, host casts to fp32.

Scales: weights are host-prescaled by 64 to lift them out of the fp8e4
subnormal range; activations carry powers-of-two scales (x: 256, P^T:
16, N1/H: 4) that cancel in the LayerNorms / relu / final constants.
"""

import numpy as np
import ml_dtypes

import concourse.bass as bass
import concourse.mybir as mybir
import concourse.tile as tile
from concourse import bacc
from concourse.bass_utils import run_bass_kernel_spmd
from concourse.masks import make_identity

P = 128          # partitions / row-tile height
D = 1024         # model dim
TH = 1024        # sequence rows per core
NT = TH // P     # 8 row tiles
KC = D // P      # 8 contraction chunks
NF = 512         # psum bank free dim (fp32)
NH = D // NF     # 2 column halves
B, T = 4, 2048
EPS_S = 0.65536  # 65536 * 1e-5: LN eps in (256x)^2 variance units
F32 = mybir.dt.float32
BF = mybir.dt.bfloat16
F8 = mybir.dt.float8e4
DR = mybir.MatmulPerfMode.DoubleRow
AF = mybir.ActivationFunctionType
ALU = mybir.AluOpType


def _build(lean=True):
    nc = bacc.Bacc(
        "TRN2", target_bir_lowering=False, debug=False, num_devices=8
    )
    x256 = nc.dram_tensor("x256", [TH, D], BF, kind="ExternalInput").ap()
    ut16 = nc.dram_tensor("ut16", [P, NT, P], BF, kind="ExternalInput").ap()
    xc2 = nc.dram_tensor("xc2", [2, NT, D], BF, kind="ExternalInput").ap()
    inv2 = nc.dram_tensor("inv2", [2, NT, P], BF, kind="ExternalInput").ap()
    Wvo8 = nc.dram_tensor("Wvo8", [P, KC, D], F8, kind="ExternalInput").ap()
    Wf18 = nc.dram_tensor("Wf18", [P, KC, D], F8, kind="ExternalInput").ap()
    Wf28 = nc.dram_tensor("Wf28", [P, KC, D], F8, kind="ExternalInput").ap()
    if not lean:
        xb256 = nc.dram_tensor("xb256", [TH, D], BF, kind="ExternalInput").ap()
        gb = nc.dram_tensor("gb", [5, D], BF, kind="ExternalInput").ap()
        fb = nc.dram_tensor("fb", [2, 2, D], BF, kind="ExternalInput").ap()
    out_bf = nc.dram_tensor("out_bf", [TH, D], BF, kind="ExternalOutput").ap()

    with tile.TileContext(nc) as tc:
        with tc.tile_pool(name="w", bufs=3) as wpool, \
             tc.tile_pool(name="cn", bufs=1) as cn, \
             tc.tile_pool(name="xs", bufs=3) as xpool, \
             tc.tile_pool(name="r1", bufs=2) as r1pool, \
             tc.tile_pool(name="nx", bufs=2) as nxpool, \
             tc.tile_pool(name="n8", bufs=2) as n8pool, \
             tc.tile_pool(name="h8", bufs=2) as h8pool, \
             tc.tile_pool(name="tp", bufs=4) as tpool, \
             tc.tile_pool(name="p8", bufs=2) as p8pool, \
             tc.tile_pool(name="z", bufs=2) as zpool, \
             tc.tile_pool(name="o", bufs=2) as opool, \
             tc.tile_pool(name="st", bufs=2) as stat, \
             tc.tile_pool(name="pmm", bufs=6, space="PSUM") as pmm, \
             tc.tile_pool(name="ptp", bufs=2, space="PSUM") as ptp:

            # ---- constants / weights ----
            ident8 = cn.tile([P, P], F8)
            make_identity(nc, ident8)
            eps_t = cn.tile([P, 1], F32)
            nc.vector.memset(eps_t, EPS_S)
            ut_sb = cn.tile([P, NT, P], BF)
            nc.sync.dma_start(out=ut_sb, in_=ut16)
            xc_sb = cn.tile([2, NT, D], BF)
            nc.sync.dma_start(out=xc_sb, in_=xc2)
            inv_sb = cn.tile([2, NT, P], BF)
            nc.sync.dma_start(out=inv_sb, in_=inv2)

            def load_w(ap, name):
                w = wpool.tile([P, KC, D], F8, tag="W", name=name)
                nc.sync.dma_start(out=w, in_=ap)
                return w

            Wvo_sb = load_w(Wvo8, "Wvo")
            Wf1_sb = load_w(Wf18, "Wf1")
            Wf2_sb = load_w(Wf28, "Wf2")

            if not lean:
                xb_sb = None  # per-tile DMA below
                g4_bc = cn.tile([P, D], BF)   # g1 * (4/256) broadcast
                nc.sync.dma_start(
                    out=g4_bc, in_=gb[0:1, :].to_broadcast([P, D]))
                g1_bc = cn.tile([P, D], BF)   # g1 broadcast
                nc.sync.dma_start(
                    out=g1_bc, in_=gb[1:2, :].to_broadcast([P, D]))
                g2_bc = cn.tile([P, D], BF)
                nc.sync.dma_start(
                    out=g2_bc, in_=gb[2:3, :].to_broadcast([P, D]))
                b2_bc = cn.tile([P, D], BF)
                nc.sync.dma_start(
                    out=b2_bc, in_=gb[3:4, :].to_broadcast([P, D]))
                b1_bc = cn.tile([P, D], BF)
                nc.sync.dma_start(
                    out=b1_bc, in_=gb[4:5, :].to_broadcast([P, D]))
                fb_sb = cn.tile([2, 2, D], BF)
                nc.sync.dma_start(out=fb_sb, in_=fb)
                ones2 = cn.tile([2, P], BF)
                nc.vector.memset(ones2[0:1, :], 1.0)
                nc.vector.memset(ones2[1:2, :], 0.0)

            def ln_coeffs(mv, big, small):
                """returns (scale_big, bias_big, scale_small, bias_small)
                for N = (r' - m')*q' with extra constant factors."""
                s = stat.tile([P, 1], F32, tag="s")
                nc.scalar.activation(
                    out=s, in_=mv[:, 1:2], func=AF.Sqrt,
                    bias=eps_t, scale=1.0,
                )
                q = stat.tile([P, 1], F32, tag="q")
                nc.vector.reciprocal(out=q, in_=s)
                qb = stat.tile([P, 1], F32, tag="qb")
                nc.vector.tensor_scalar_mul(out=qb, in0=q, scalar1=float(big))
                mbb = stat.tile([P, 1], F32, tag="mbb")
                nc.vector.tensor_scalar(
                    out=mbb, in0=mv[:, 0:1], scalar1=qb, scalar2=-1.0,
                    op0=ALU.mult, op1=ALU.mult,
                )
                if small is None:
                    return qb, mbb, None, None
                qs = stat.tile([P, 1], F32, tag="qs")
                nc.vector.tensor_scalar_mul(out=qs, in0=q, scalar1=float(small))
                mbs = stat.tile([P, 1], F32, tag="mbs")
                nc.vector.tensor_scalar(
                    out=mbs, in0=mv[:, 0:1], scalar1=qs, scalar2=-1.0,
                    op0=ALU.mult, op1=ALU.mult,
                )
                return qb, mbb, qs, mbs

            def bn(src, tag):
                st = stat.tile([P, NH, 6], F32, tag=f"st{tag}")
                for h in range(NH):
                    nc.vector.bn_stats(
                        out=st[:, h, :], in_=src[:, h * NF:(h + 1) * NF])
                mv = stat.tile([P, 2], F32, tag=f"mv{tag}")
                nc.vector.bn_aggr(out=mv, in_=st)
                return mv

            for j in range(NT):
                jsl = slice(j * P, (j + 1) * P)
                x_t = xpool.tile([P, D], BF, tag="x", name="x")
                nc.sync.dma_start(out=x_t, in_=x256[jsl, :])
                if not lean:
                    xb_t = xpool.tile([P, D], BF, tag="xb", name="xb")
                    nc.sync.dma_start(out=xb_t, in_=xb256[jsl, :])

                # ---- prefix-mean, transposed: pt[d, t] = 4096 * P^T ----
                pts = [pmm.tile([P, 4, P], F32, tag="mm", name=f"pt{g}")
                       for g in range(2)]
                for kc in range(KC):
                    sl = pts[kc // 4][:, kc % 4, :]
                    ksl = slice(kc * P, (kc + 1) * P)
                    nc.tensor.matmul(
                        sl, lhsT=x_t[:, ksl], rhs=ut_sb[:, j, :],
                        start=True, stop=False,
                    )
                    nc.tensor.matmul(
                        sl, lhsT=xc_sb[:, j, ksl], rhs=inv_sb[:, j, :],
                        start=False, stop=True,
                    )
                pt8 = p8pool.tile([P, KC, P], F8, tag="p8", name="PT8")
                for g in range(2):
                    nc.scalar.activation(
                        out=pt8[:, 4 * g:4 * g + 4, :], in_=pts[g],
                        func=AF.Copy, scale=1.0 / 256.0,
                    )

                # ---- AO = 16P @ 64Wvo = 1024*AO;  r1 = 256*(AO + x) ----
                r1 = r1pool.tile([P, D], BF, tag="r1", name="r1")
                for n in range(NH):
                    nsl = slice(n * NF, (n + 1) * NF)
                    ao = pmm.tile([P, NF], F32, tag="mm")
                    for g in range(4):
                        gsl = slice(2 * g, 2 * g + 2)
                        nc.tensor.matmul(
                            ao, lhsT=pt8[:, gsl, :],
                            rhs=Wvo_sb[:, gsl, nsl],
                            start=(g == 0), stop=(g == 3), perf_mode=DR,
                        )
                    nc.vector.scalar_tensor_tensor(
                        out=r1[:, nsl], in0=ao, scalar=0.25,
                        in1=(x_t if lean else xb_t)[:, nsl],
                        op0=ALU.mult, op1=ALU.add,
                    )

                # ---- LN1 ----
                mv1 = bn(r1, "1")
                q256, mb256, q4, mb4 = ln_coeffs(mv1, 256, 4)
                n1x = nxpool.tile([P, D], BF, tag="nx", name="N1x")
                n18 = n8pool.tile([P, D], F8, tag="n8", name="N18")
                if lean:
                    # N1x = 256*(N1 + x); N18 = 4*N1
                    nc.vector.affine_then_add(
                        out=n1x, in0=r1, in1=x_t, scale=q256, bias=mb256)
                    nc.scalar.activation(
                        out=n18, in_=r1, func=AF.Identity,
                        bias=mb4, scale=q4,
                    )
                else:
                    n1g = nxpool.tile([P, D], BF, tag="ng", name="N1g")
                    nc.scalar.activation(
                        out=n1g, in_=r1, func=AF.Identity,
                        bias=mb256, scale=q256,
                    )
                    nc.vector.tensor_mul(out=n18, in0=n1g, in1=g4_bc)
                    # n1x = 256*(N1*g1 + b1 + x)
                    nc.vector.tensor_mul(out=n1g, in0=n1g, in1=g1_bc)
                    nc.vector.tensor_add(out=n1g, in0=n1g, in1=b1_bc)
                    nc.vector.tensor_add(out=n1x, in0=n1g, in1=x_t)

                # ---- FFN1: H = relu(4N1 @ 64Wf1)/64 = 4*relu(N1@Wf1) ----
                t1 = ptp.tile([P, KC, P, 2], F8, tag="tp")
                for kc in range(KC):
                    nc.tensor.transpose(
                        t1[:, kc, :, 0], n18[:, kc * P:(kc + 1) * P], ident8)
                n1t = tpool.tile([P, KC, P], F8, tag="t", name="N1T")
                nc.vector.tensor_copy(out=n1t, in_=t1[:, :, :, 0])

                h8 = h8pool.tile([P, D], F8, tag="h8", name="H8")
                for n in range(NH):
                    nsl = slice(n * NF, (n + 1) * NF)
                    f1 = pmm.tile([P, NF], F32, tag="mm")
                    for g in range(4):
                        gsl = slice(2 * g, 2 * g + 2)
                        nc.tensor.matmul(
                            f1, lhsT=n1t[:, gsl, :],
                            rhs=Wf1_sb[:, gsl, nsl],
                            start=(g == 0), stop=(g == 3 and lean),
                            perf_mode=DR,
                        )
                    if not lean:
                        nc.tensor.matmul(
                            f1, lhsT=ones2, rhs=fb_sb[:, 0, nsl],
                            start=False, stop=True,
                        )
                    nc.scalar.activation(
                        out=h8[:, nsl], in_=f1, func=AF.Relu,
                        scale=1.0 / 64.0,
                    )

                # ---- FFN2: z = 256*(ff + N1 + x) ----
                t2 = ptp.tile([P, KC, P, 2], F8, tag="tp")
                for kc in range(KC):
                    nc.tensor.transpose(
                        t2[:, kc, :, 0], h8[:, kc * P:(kc + 1) * P], ident8)
                ht = tpool.tile([P, KC, P], F8, tag="t", name="HT")
                nc.vector.tensor_copy(out=ht, in_=t2[:, :, :, 0])

                z = zpool.tile([P, D], BF, tag="z", name="z")
                for n in range(NH):
                    nsl = slice(n * NF, (n + 1) * NF)
                    f2 = pmm.tile([P, NF], F32, tag="mm")
                    for g in range(4):
                        gsl = slice(2 * g, 2 * g + 2)
                        nc.tensor.matmul(
                            f2, lhsT=ht[:, gsl, :],
                            rhs=Wf2_sb[:, gsl, nsl],
                            start=(g == 0), stop=(g == 3 and lean),
                            perf_mode=DR,
                        )
                    if not lean:
                        nc.tensor.matmul(
                            f2, lhsT=ones2, rhs=fb_sb[:, 1, nsl],
                            start=False, stop=True,
                        )
                    nc.vector.tensor_add(
                        out=z[:, nsl], in0=f2, in1=n1x[:, nsl])

                # ---- LN2 -> out ----
                mv2 = bn(z, "2")
                q2, mb2, _, _ = ln_coeffs(mv2, 1, None)
                o = opool.tile([P, D], BF, tag="o", name="o")
                if lean:
                    nc.scalar.activation(
                        out=o, in_=z, func=AF.Identity, bias=mb2, scale=q2)
                else:
                    op = opool.tile([P, D], BF, tag="op", name="op")
                    nc.scalar.activation(
                        out=op, in_=z, func=AF.Identity, bias=mb2, scale=q2)
                    nc.vector.tensor_mul(out=op, in0=op, in1=g2_bc)
                    nc.vector.tensor_add(out=o, in0=op, in1=b2_bc)
                nc.sync.dma_start(out=out_bf[jsl, :], in_=o)

    nc.compile()
    return nc


_CACHE = {}


def _get_nc(lean=True):
    key = "lean" if lean else "general"
    if key not in _CACHE:
        _CACHE[key] = _build(lean=lean)
    return _CACHE[key]


BF_NP = ml_dtypes.bfloat16
F8_NP = ml_dtypes.float8_e4m3


def _in_maps(x, Wv, Wo, bo, g1, b1, Wf1, bf1, Wf2, bf2, g2, b2, lean=True):
    x = np.asarray(x, dtype=np.float32)
    Wv_all = np.asarray(Wv, np.float32).transpose(1, 0, 2).reshape(D, D)
    Wvo = Wv_all @ np.asarray(Wo, np.float32)

    def wprep(w):
        # [D, D] -> [P, KC, D] so each partition's SBUF row is contiguous
        w8 = np.asarray(64.0 * w, F8_NP)
        return np.ascontiguousarray(
            w8.reshape(KC, P, D).transpose(1, 0, 2))

    base = {
        "Wvo8": wprep(Wvo),
        "Wf18": wprep(np.asarray(Wf1, np.float32)),
        "Wf28": wprep(np.asarray(Wf2, np.float32)),
    }
    if not lean:
        base["gb"] = np.asarray(np.stack([
            np.asarray(g1, np.float32) / 64.0,   # n18 = (256*N1) * g1/64
            np.asarray(g1, np.float32),
            np.asarray(g2, np.float32),
            np.asarray(b2, np.float32),
            256.0 * np.asarray(b1, np.float32),
        ]), BF_NP)
        base["fb"] = np.asarray(np.stack([
            np.stack([256.0 * (np.asarray(b1, np.float32) @
                               np.asarray(Wf1, np.float32) +
                               np.asarray(bf1, np.float32)),
                      np.zeros(D, np.float32)]),
            np.stack([256.0 * np.asarray(bf2, np.float32),
                      np.zeros(D, np.float32)]),
        ]), BF_NP)

    # per-half triangular / inv-count tables
    uts, invs = [], []
    for half in range(2):
        t0 = half * TH
        ut = np.zeros((P, NT, P), np.float32)
        iv = np.zeros((2, NT, P), np.float32)
        for j in range(NT):
            cnt = (t0 + j * P + np.arange(P) + 1.0).astype(np.float32)
            ut[:, j, :] = np.triu(np.ones((P, P), np.float32)) * (16.0 / cnt)
            iv[0, j, :] = 16.0 / cnt
        uts.append(np.asarray(ut, BF_NP))
        invs.append(np.asarray(iv, BF_NP))

    in_maps = []
    for c in range(8):
        b, half = divmod(c, 2)
        t0 = half * TH
        m = dict(base)
        xh = x[b, t0:t0 + TH]
        m["x256"] = np.asarray(256.0 * xh, BF_NP)
        if not lean:
            m["xb256"] = np.asarray(
                256.0 * (xh + np.asarray(bo, np.float32)), BF_NP)
        # cumulative column sums before each tile (global within batch)
        ts_sums = x[b].reshape(2 * NT, P, D).sum(axis=1, dtype=np.float64)
        starts = np.zeros((2 * NT, D), np.float64)
        starts[1:] = np.cumsum(ts_sums[:-1], axis=0)
        xc = np.zeros((2, NT, D), np.float32)
        xc[0] = 256.0 * starts[half * NT:(half + 1) * NT].astype(np.float32)
        m["xc2"] = np.asarray(xc, BF_NP)
        m["ut16"] = uts[half]
        m["inv2"] = invs[half]
        in_maps.append(m)
    return in_maps


def _assemble(results):
    out = np.empty((B, T, D), np.float32)
    for c in range(8):
        b, half = divmod(c, 2)
        out[b, half * TH:(half + 1) * TH] = (
            results[c]["out_bf"].astype(np.float32))
    return out


def kernel(x, Wk, Wv, Wo, bo, g1, b1, Wf1, bf1, Wf2, bf2, g2, b2):
    lean = bool(
        not np.any(np.asarray(bo)) and not np.any(np.asarray(bf1))
        and not np.any(np.asarray(bf2)) and not np.any(np.asarray(b1))
        and not np.any(np.asarray(b2))
        and np.all(np.asarray(g1) == 1.0) and np.all(np.asarray(g2) == 1.0)
    )
    in_maps = _in_maps(
        x, Wv, Wo, bo, g1, b1, Wf1, bf1, Wf2, bf2, g2, b2, lean=lean)
    res = run_bass_kernel_spmd(_get_nc(lean), in_maps, list(range(8))).results
    return _assemble(res)


# revision 18
# speedup vs baseline: 1.6418x; 1.0471x over previous
"""TRN2 8-core SPMD kernel for nn_DecoderBlock_13443247636967 (v2).

Math (validated vs fp32 reference in numpy, rel err ~8e-3):
the reference's softmax scale HS**-5 = 2**-30 makes every pre-softmax
score < 4e-8, so softmax is exactly the uniform causal average at fp32
resolution and attention reduces to a causal prefix-mean of V.  Since
prefix-mean is linear, attn_out = prefix_mean(x) @ (Wv @ Wo): the value
and output projections fold into one host-precomputed matrix Wvo and V
is never materialized.  Wk cannot affect the output at fp32 resolution.

Device pipeline per 128-row tile (core c = batch c//2, half c%2):
  P^T   = matmul(lhsT=x-chunk, rhs=scaled-upper-tri) + K=2 rank-1 carry
          (prefix-mean computed directly in transposed orientation; the
          carry row is the host-computed cumulative column-sum of x)
  AO    = P^T(fp8) @ Wvo(fp8)        [DoubleRow, 2x PE rate]
  r1    = AO*0.25 + x256             [fused scalar_tensor_tensor]
  N1    = LN1(r1)                    [bn_stats; scale-invariant => the
                                      x256 = 256*x scaling cancels]
  H     = relu(N1T(fp8) @ Wf1(fp8))  [DoubleRow; relu absorbs scales]
  z     = HT(fp8) @ Wf2(fp8) + (N1 + x)
  out   = LN2(z) -> bf16, host casts to fp32.

Scales: weights are host-prescaled by 64 to lift them out of the fp8e4
subnormal range; activations carry powers-of-two scales (x: 256, P^T:
16, N1/H: 4) that cancel in the LayerNorms / relu / final constants.
"""

import numpy as np
import ml_dtypes

import concourse.bass as bass
import concourse.mybir as mybir
import concourse.tile as tile
from concourse import bacc
from concourse.bass_utils import run_bass_kernel_spmd
from concourse.masks import make_identity

P = 128          # partitions / row-tile height
D = 1024         # model dim
TH = 1024        # sequence rows per core
NT = TH // P     # 8 row tiles
KC = D // P      # 8 contraction chunks
NF = 512         # psum bank free dim (fp32)
NH = D // NF     # 2 column halves
B, T = 4, 2048
EPS_S = 0.65536  # 65536 * 1e-5: LN eps in (256x)^2 variance units
F32 = mybir.dt.float32
BF = mybir.dt.bfloat16
F8 = mybir.dt.float8e4
DR = mybir.MatmulPerfMode.DoubleRow
AF = mybir.ActivationFunctionType
ALU = mybir.AluOpType


def _build(lean=True):
    nc = bacc.Bacc(
        "TRN2", target_bir_lowering=False, debug=False, num_devices=8
    )
    x256 = nc.dram_tensor("x256", [TH, D], BF, kind="ExternalInput").ap()
    ut16 = nc.dram_tensor("ut16", [P, NT, P], BF, kind="ExternalInput").ap()
    xc2 = nc.dram_tensor("xc2", [2, NT, D], BF, kind="ExternalInput").ap()
    inv2 = nc.dram_tensor("inv2", [2, NT, P], BF, kind="ExternalInput").ap()
    Wvo8 = nc.dram_tensor("Wvo8", [P, KC, D], F8, kind="ExternalInput").ap()
    Wf18 = nc.dram_tensor("Wf18", [P, KC, D], F8, kind="ExternalInput").ap()
    Wf28 = nc.dram_tensor("Wf28", [P, KC, D], F8, kind="ExternalInput").ap()
    if not lean:
        xb256 = nc.dram_tensor("xb256", [TH, D], BF, kind="ExternalInput").ap()
        gb = nc.dram_tensor("gb", [5, D], BF, kind="ExternalInput").ap()
        fb = nc.dram_tensor("fb", [2, 2, D], BF, kind="ExternalInput").ap()
    out_bf = nc.dram_tensor("out_bf", [TH, D], BF, kind="ExternalOutput").ap()

    with tile.TileContext(nc) as tc:
        with tc.tile_pool(name="w", bufs=3) as wpool, \
             tc.tile_pool(name="cn", bufs=1) as cn, \
             tc.tile_pool(name="xs", bufs=3) as xpool, \
             tc.tile_pool(name="r1", bufs=2) as r1pool, \
             tc.tile_pool(name="nx", bufs=2) as nxpool, \
             tc.tile_pool(name="n8", bufs=2) as n8pool, \
             tc.tile_pool(name="h8", bufs=2) as h8pool, \
             tc.tile_pool(name="tp", bufs=4) as tpool, \
             tc.tile_pool(name="p8", bufs=2) as p8pool, \
             tc.tile_pool(name="z", bufs=2) as zpool, \
             tc.tile_pool(name="o", bufs=2) as opool, \
             tc.tile_pool(name="st", bufs=2) as stat, \
             tc.tile_pool(name="pfx", bufs=1, space="PSUM") as pfx, \
             tc.tile_pool(name="pao", bufs=2, space="PSUM") as pao, \
             tc.tile_pool(name="pf", bufs=2, space="PSUM") as pf, \
             tc.tile_pool(name="ptp", bufs=2, space="PSUM") as ptp:

            # ---- constants / weights ----
            identb = cn.tile([P, P], BF)
            make_identity(nc, identb)
            eps_t = cn.tile([P, 1], F32)
            nc.vector.memset(eps_t, EPS_S)
            ut_sb = cn.tile([P, NT, P], BF)
            nc.sync.dma_start(out=ut_sb, in_=ut16)
            xc_sb = cn.tile([2, NT, D], BF)
            nc.sync.dma_start(out=xc_sb, in_=xc2)
            inv_sb = cn.tile([2, NT, P], BF)
            nc.sync.dma_start(out=inv_sb, in_=inv2)

            def load_w(ap, name):
                w = wpool.tile([P, KC, D], F8, tag="W", name=name)
                nc.sync.dma_start(out=w, in_=ap)
                return w

            Wvo_sb = load_w(Wvo8, "Wvo")
            Wf1_sb = load_w(Wf18, "Wf1")
            Wf2_sb = load_w(Wf28, "Wf2")

            if not lean:
                xb_sb = None  # per-tile DMA below
                g4_bc = cn.tile([P, D], BF)   # g1 * (4/256) broadcast
                nc.sync.dma_start(
                    out=g4_bc, in_=gb[0:1, :].to_broadcast([P, D]))
                g1_bc = cn.tile([P, D], BF)   # g1 broadcast
                nc.sync.dma_start(
                    out=g1_bc, in_=gb[1:2, :].to_broadcast([P, D]))
                g2_bc = cn.tile([P, D], BF)
                nc.sync.dma_start(
                    out=g2_bc, in_=gb[2:3, :].to_broadcast([P, D]))
                b2_bc = cn.tile([P, D], BF)
                nc.sync.dma_start(
                    out=b2_bc, in_=gb[3:4, :].to_broadcast([P, D]))
                b1_bc = cn.tile([P, D], BF)
                nc.sync.dma_start(
                    out=b1_bc, in_=gb[4:5, :].to_broadcast([P, D]))
                fb_sb = cn.tile([2, 2, D], BF)
                nc.sync.dma_start(out=fb_sb, in_=fb)
                ones2 = cn.tile([2, P], BF)
                nc.vector.memset(ones2[0:1, :], 1.0)
                nc.vector.memset(ones2[1:2, :], 0.0)

            def ln_coeffs(mv, big, small):
                """returns (scale_big, bias_big, scale_small, bias_small)
                for N = (r' - m')*q' with extra constant factors."""
                s = stat.tile([P, 1], F32, tag="s")
                nc.scalar.activation(
                    out=s, in_=mv[:, 1:2], func=AF.Sqrt,
                    bias=eps_t, scale=1.0,
                )
                q = stat.tile([P, 1], F32, tag="q")
                nc.vector.reciprocal(out=q, in_=s)
                qb = stat.tile([P, 1], F32, tag="qb")
                nc.vector.tensor_scalar_mul(out=qb, in0=q, scalar1=float(big))
                mbb = stat.tile([P, 1], F32, tag="mbb")
                nc.vector.tensor_scalar(
                    out=mbb, in0=mv[:, 0:1], scalar1=qb, scalar2=-1.0,
                    op0=ALU.mult, op1=ALU.mult,
                )
                if small is None:
                    return qb, mbb, None, None
                qs = stat.tile([P, 1], F32, tag="qs")
                nc.vector.tensor_scalar_mul(out=qs, in0=q, scalar1=float(small))
                mbs = stat.tile([P, 1], F32, tag="mbs")
                nc.vector.tensor_scalar(
                    out=mbs, in0=mv[:, 0:1], scalar1=qs, scalar2=-1.0,
                    op0=ALU.mult, op1=ALU.mult,
                )
                return qb, mbb, qs, mbs

            def bn(src, tag):
                st = stat.tile([P, NH, 6], F32, tag=f"st{tag}")
                for h in range(NH):
                    nc.vector.bn_stats(
                        out=st[:, h, :], in_=src[:, h * NF:(h + 1) * NF])
                mv = stat.tile([P, 2], F32, tag=f"mv{tag}")
                nc.vector.bn_aggr(out=mv, in_=st)
                return mv

            for j in range(NT):
                jsl = slice(j * P, (j + 1) * P)
                x_t = xpool.tile([P, D], BF, tag="x", name="x")
                nc.sync.dma_start(out=x_t, in_=x256[jsl, :])
                if not lean:
                    xb_t = xpool.tile([P, D], BF, tag="xb", name="xb")
                    nc.sync.dma_start(out=xb_t, in_=xb256[jsl, :])

                # ---- prefix-mean, transposed: pt[d, t] = 4096 * P^T ----
                pts = pfx.tile([P, KC, P], F32, tag="pf", name="pts")
                for kc in range(KC):
                    sl = pts[:, kc, :]
                    ksl = slice(kc * P, (kc + 1) * P)
                    nc.tensor.matmul(
                        sl, lhsT=x_t[:, ksl], rhs=ut_sb[:, j, :],
                        start=True, stop=False,
                    )
                    nc.tensor.matmul(
                        sl, lhsT=xc_sb[:, j, ksl], rhs=inv_sb[:, j, :],
                        start=False, stop=True,
                    )
                pt8 = p8pool.tile([P, KC, P], F8, tag="p8", name="PT8")
                nc.scalar.activation(
                    out=pt8, in_=pts, func=AF.Copy, scale=1.0 / 256.0,
                )

                # ---- AO = 16P @ 64Wvo = 1024*AO;  r1 = 256*(AO + x) ----
                r1 = r1pool.tile([P, D], BF, tag="r1", name="r1")
                for n in range(NH):
                    nsl = slice(n * NF, (n + 1) * NF)
                    ao = pao.tile([P, NF], F32, tag="ao")
                    for g in range(4):
                        gsl = slice(2 * g, 2 * g + 2)
                        nc.tensor.matmul(
                            ao, lhsT=pt8[:, gsl, :],
                            rhs=Wvo_sb[:, gsl, nsl],
                            start=(g == 0), stop=(g == 3), perf_mode=DR,
                        )
                    nc.vector.scalar_tensor_tensor(
                        out=r1[:, nsl], in0=ao, scalar=0.25,
                        in1=(x_t if lean else xb_t)[:, nsl],
                        op0=ALU.mult, op1=ALU.add,
                    )

                # ---- LN1 ----
                mv1 = bn(r1, "1")
                q256, mb256, q4, mb4 = ln_coeffs(mv1, 256, 4)
                n1x = nxpool.tile([P, D], BF, tag="nx", name="N1x")
                n18 = n8pool.tile([P, D], BF, tag="n8", name="N18")
                if lean:
                    # N1x = 256*(N1 + x); N18 = 4*N1
                    nc.vector.affine_then_add(
                        out=n1x, in0=r1, in1=x_t, scale=q256, bias=mb256)
                    nc.scalar.activation(
                        out=n18, in_=r1, func=AF.Identity,
                        bias=mb4, scale=q4,
                    )
                else:
                    n1g = nxpool.tile([P, D], BF, tag="ng", name="N1g")
                    nc.scalar.activation(
                        out=n1g, in_=r1, func=AF.Identity,
                        bias=mb256, scale=q256,
                    )
                    nc.vector.tensor_mul(out=n18, in0=n1g, in1=g4_bc)
                    # n1x = 256*(N1*g1 + b1 + x)
                    nc.vector.tensor_mul(out=n1g, in0=n1g, in1=g1_bc)
                    nc.vector.tensor_add(out=n1g, in0=n1g, in1=b1_bc)
                    nc.vector.tensor_add(out=n1x, in0=n1g, in1=x_t)

                # ---- FFN1: H = relu(4N1 @ 64Wf1)/64 = 4*relu(N1@Wf1) ----
                t1 = ptp.tile([P, KC, P], BF, tag="tp")
                for kc in range(KC):
                    nc.tensor.transpose(
                        t1[:, kc, :], n18[:, kc * P:(kc + 1) * P], identb)
                n1t = tpool.tile([P, KC, P], F8, tag="t", name="N1T")
                nc.scalar.activation(out=n1t, in_=t1, func=AF.Copy)

                h8 = h8pool.tile([P, D], BF, tag="h8", name="H8")
                for n in range(NH):
                    nsl = slice(n * NF, (n + 1) * NF)
                    f1 = pf.tile([P, NF], F32, tag="f")
                    for g in range(4):
                        gsl = slice(2 * g, 2 * g + 2)
                        nc.tensor.matmul(
                            f1, lhsT=n1t[:, gsl, :],
                            rhs=Wf1_sb[:, gsl, nsl],
                            start=(g == 0), stop=(g == 3 and lean),
                            perf_mode=DR,
                        )
                    if not lean:
                        nc.tensor.matmul(
                            f1, lhsT=ones2, rhs=fb_sb[:, 0, nsl],
                            start=False, stop=True,
                        )
                    nc.scalar.activation(
                        out=h8[:, nsl], in_=f1, func=AF.Relu,
                        scale=1.0 / 64.0,
                    )

                # ---- FFN2: z = 256*(ff + N1 + x) ----
                t2 = ptp.tile([P, KC, P], BF, tag="tp")
                for kc in range(KC):
                    nc.tensor.transpose(
                        t2[:, kc, :], h8[:, kc * P:(kc + 1) * P], identb)
                ht = tpool.tile([P, KC, P], F8, tag="t", name="HT")
                nc.vector.tensor_copy(out=ht, in_=t2)

                z = zpool.tile([P, D], BF, tag="z", name="z")
                for n in range(NH):
                    nsl = slice(n * NF, (n + 1) * NF)
                    f2 = pf.tile([P, NF], F32, tag="f")
                    for g in range(4):
                        gsl = slice(2 * g, 2 * g + 2)
                        nc.tensor.matmul(
                            f2, lhsT=ht[:, gsl, :],
                            rhs=Wf2_sb[:, gsl, nsl],
                            start=(g == 0), stop=(g == 3 and lean),
                            perf_mode=DR,
                        )
                    if not lean:
                        nc.tensor.matmul(
                            f2, lhsT=ones2, rhs=fb_sb[:, 1, nsl],
                            start=False, stop=True,
                        )
                    nc.vector.tensor_add(
                        out=z[:, nsl], in0=f2, in1=n1x[:, nsl])

                # ---- LN2 -> out ----
                mv2 = bn(z, "2")
                q2, mb2, _, _ = ln_coeffs(mv2, 1, None)
                o = opool.tile([P, D], BF, tag="o", name="o")
                if lean:
                    nc.scalar.activation(
                        out=o, in_=z, func=AF.Identity, bias=mb2, scale=q2)
                else:
                    op = opool.tile([P, D], BF, tag="op", name="op")
                    nc.scalar.activation(
                        out=op, in_=z, func=AF.Identity, bias=mb2, scale=q2)
                    nc.vector.tensor_mul(out=op, in0=op, in1=g2_bc)
                    nc.vector.tensor_add(out=o, in0=op, in1=b2_bc)
                nc.sync.dma_start(out=out_bf[jsl, :], in_=o)

    nc.compile()
    return nc


_CACHE = {}


def _get_nc(lean=True):
    key = "lean" if lean else "general"
    if key not in _CACHE:
        _CACHE[key] = _build(lean=lean)
    return _CACHE[key]


BF_NP = ml_dtypes.bfloat16
F8_NP = ml_dtypes.float8_e4m3


def _in_maps(x, Wv, Wo, bo, g1, b1, Wf1, bf1, Wf2, bf2, g2, b2, lean=True):
    x = np.asarray(x, dtype=np.float32)
    Wv_all = np.asarray(Wv, np.float32).transpose(1, 0, 2).reshape(D, D)
    Wvo = Wv_all @ np.asarray(Wo, np.float32)

    def wprep(w):
        # [D, D] -> [P, KC, D] so each partition's SBUF row is contiguous
        w8 = np.asarray(64.0 * w, F8_NP)
        return np.ascontiguousarray(
            w8.reshape(KC, P, D).transpose(1, 0, 2))

    base = {
        "Wvo8": wprep(Wvo),
        "Wf18": wprep(np.asarray(Wf1, np.float32)),
        "Wf28": wprep(np.asarray(Wf2, np.float32)),
    }
    if not lean:
        base["gb"] = np.asarray(np.stack([
            np.asarray(g1, np.float32) / 64.0,   # n18 = (256*N1) * g1/64
            np.asarray(g1, np.float32),
            np.asarray(g2, np.float32),
            np.asarray(b2, np.float32),
            256.0 * np.asarray(b1, np.float32),
        ]), BF_NP)
        base["fb"] = np.asarray(np.stack([
            np.stack([256.0 * (np.asarray(b1, np.float32) @
                               np.asarray(Wf1, np.float32) +
                               np.asarray(bf1, np.float32)),
                      np.zeros(D, np.float32)]),
            np.stack([256.0 * np.asarray(bf2, np.float32),
                      np.zeros(D, np.float32)]),
        ]), BF_NP)

    # per-half triangular / inv-count tables
    uts, invs = [], []
    for half in range(2):
        t0 = half * TH
        ut = np.zeros((P, NT, P), np.float32)
        iv = np.zeros((2, NT, P), np.float32)
        for j in range(NT):
            cnt = (t0 + j * P + np.arange(P) + 1.0).astype(np.float32)
            ut[:, j, :] = np.triu(np.ones((P, P), np.float32)) * (16.0 / cnt)
            iv[0, j, :] = 16.0 / cnt
        uts.append(np.asarray(ut, BF_NP))
        invs.append(np.asarray(iv, BF_NP))

    in_maps = []
    for c in range(8):
        b, half = divmod(c, 2)
        t0 = half * TH
        m = dict(base)
        xh = x[b, t0:t0 + TH]
        m["x256"] = np.asarray(256.0 * xh, BF_NP)
        if not lean:
            m["xb256"] = np.asarray(
                256.0 * (xh + np.asarray(bo, np.float32)), BF_NP)
        # cumulative column sums before each tile (global within batch)
        ts_sums = x[b].reshape(2 * NT, P, D).sum(axis=1, dtype=np.float64)
        starts = np.zeros((2 * NT, D), np.float64)
        starts[1:] = np.cumsum(ts_sums[:-1], axis=0)
        xc = np.zeros((2, NT, D), np.float32)
        xc[0] = 256.0 * starts[half * NT:(half + 1) * NT].astype(np.float32)
        m["xc2"] = np.asarray(xc, BF_NP)
        m["ut16"] = uts[half]
        m["inv2"] = invs[half]
        in_maps.append(m)
    return in_maps


def _assemble(results):
    out = np.empty((B, T, D), np.float32)
    for c in range(8):
        b, half = divmod(c, 2)
        out[b, half * TH:(half + 1) * TH] = (
            results[c]["out_bf"].astype(np.float32))
    return out


def kernel(x, Wk, Wv, Wo, bo, g1, b1, Wf1, bf1, Wf2, bf2, g2, b2):
    lean = bool(
        not np.any(np.asarray(bo)) and not np.any(np.asarray(bf1))
        and not np.any(np.asarray(bf2)) and not np.any(np.asarray(b1))
        and not np.any(np.asarray(b2))
        and np.all(np.asarray(g1) == 1.0) and np.all(np.asarray(g2) == 1.0)
    )
    in_maps = _in_maps(
        x, Wv, Wo, bo, g1, b1, Wf1, bf1, Wf2, bf2, g2, b2, lean=lean)
    res = run_bass_kernel_spmd(_get_nc(lean), in_maps, list(range(8))).results
    return _assemble(res)


# revision 19
# speedup vs baseline: 3.1084x; 1.8933x over previous
"""TRN2 8-core SPMD kernel for nn_DecoderBlock_13443247636967 (v4).

Math (validated vs fp32 reference in numpy, rel err ~8.5e-3):
the reference's softmax scale HS**-5 = 2**-30 makes every pre-softmax
score < 4e-8, so softmax is exactly the uniform causal average at fp32
resolution and attention reduces to a causal prefix-mean of V.  Since
prefix-mean is linear, attn_out = prefix_mean(x) @ (Wv @ Wo): the value
and output projections fold into one host-precomputed matrix Wvo and V
is never materialized.  Wk cannot affect the output at fp32 resolution.

Device pipeline per 128-row tile (core c = batch c//2, half c%2):
  P^T = matmul(lhsT=xp-chunk, rhs=count-scaled-upper-tri)  [8 matmuls;
        the cross-tile carry (cumulative column sum of x) is folded by
        the host into row 0 of xp, which the inclusive triangular
        matrix then propagates to every row]
  AO  = P^T(fp8) @ Wvo(fp8)            [DoubleRow, 2x PE rate]
  r1  = AO*0.25 + x256                 [fused scalar_tensor_tensor]
  N1  = LN1(r1)  (scale-invariant => the 256x scaling cancels)
  H   = relu(N1T(fp8) @ Wf1(fp8))      [DoubleRow]
  z   = HT(fp8) @ Wf2(fp8) + (N1 + x)  [N1+x add runs on idle GpSimd]
  out = LN2(z) -> bf16, host casts to fp32.

Scales: weights host-prescaled by 64 (lifts them out of fp8e4
subnormals); activations carry powers-of-two scales (x: 256, P^T: 16,
N1T: 16, H: 4) folded into the PSUM->SBUF cast constants and LayerNorm
coefficient algebra.  Transposes run on the PE in bf16; quantization
to fp8 happens in the contiguous PSUM->SBUF cast.
"""

import numpy as np
import ml_dtypes

import concourse.bass as bass
import concourse.mybir as mybir
import concourse.tile as tile
from concourse import bacc
from concourse.bass_utils import run_bass_kernel_spmd
from concourse.masks import make_identity

P = 128          # partitions / row-tile height
D = 1024         # model dim
TH = 1024        # sequence rows per core
NT = TH // P     # 8 row tiles
KC = D // P      # 8 contraction chunks
NF = 512         # psum bank free dim (fp32)
NH = D // NF     # 2 column halves
B, T = 4, 2048
F32 = mybir.dt.float32
BF = mybir.dt.bfloat16
F8 = mybir.dt.float8e4
DR = mybir.MatmulPerfMode.DoubleRow
AF = mybir.ActivationFunctionType
ALU = mybir.AluOpType


def _build(lean=True):
    nc = bacc.Bacc(
        "TRN2", target_bir_lowering=False, debug=False, num_devices=8
    )
    x256 = nc.dram_tensor("x256", [TH, D], BF, kind="ExternalInput").ap()
    xp256 = nc.dram_tensor("xp256", [TH, D], BF, kind="ExternalInput").ap()
    ut16 = nc.dram_tensor("ut16", [P, NT, P], BF, kind="ExternalInput").ap()
    Wvo8 = nc.dram_tensor("Wvo8", [P, KC, D], F8, kind="ExternalInput").ap()
    Wf18 = nc.dram_tensor("Wf18", [P, KC, D], F8, kind="ExternalInput").ap()
    Wf28 = nc.dram_tensor("Wf28", [P, KC, D], F8, kind="ExternalInput").ap()
    if not lean:
        xb256 = nc.dram_tensor("xb256", [TH, D], BF, kind="ExternalInput").ap()
        gb = nc.dram_tensor("gb", [4, D], BF, kind="ExternalInput").ap()
        fb = nc.dram_tensor("fb", [2, 2, D], BF, kind="ExternalInput").ap()
    out_bf = nc.dram_tensor("out_bf", [TH, D], BF, kind="ExternalOutput").ap()

    with tile.TileContext(nc) as tc:
        with tc.tile_pool(name="w", bufs=3) as wpool, \
             tc.tile_pool(name="cn", bufs=1) as cn, \
             tc.tile_pool(name="xs", bufs=3) as xpool, \
             tc.tile_pool(name="xp", bufs=3) as xppool, \
             tc.tile_pool(name="r1", bufs=3) as r1pool, \
             tc.tile_pool(name="nx", bufs=3) as nxpool, \
             tc.tile_pool(name="n8", bufs=3) as n8pool, \
             tc.tile_pool(name="h8", bufs=3) as h8pool, \
             tc.tile_pool(name="tp", bufs=4) as tpool, \
             tc.tile_pool(name="p8", bufs=3) as p8pool, \
             tc.tile_pool(name="z", bufs=3) as zpool, \
             tc.tile_pool(name="o", bufs=3) as opool, \
             tc.tile_pool(name="st", bufs=3) as stat, \
             tc.tile_pool(name="pfx", bufs=2, space="PSUM") as pfx, \
             tc.tile_pool(name="pao", bufs=2, space="PSUM") as pao, \
             tc.tile_pool(name="pf", bufs=2, space="PSUM") as pf, \
             tc.tile_pool(name="ptp", bufs=2, space="PSUM") as ptp:

            # ---- constants / weights ----
            identb = cn.tile([P, P], BF)
            make_identity(nc, identb)
            eps1 = cn.tile([P, 1], F32)
            nc.vector.memset(eps1, 1e-5)
            eps2 = cn.tile([P, 1], F32)
            nc.vector.memset(eps2, 0.65536)
            ut_sb = cn.tile([P, NT, P], BF)
            nc.sync.dma_start(out=ut_sb, in_=ut16)

            def load_w(ap, name):
                w = wpool.tile([P, KC, D], F8, tag="W", name=name)
                nc.sync.dma_start(out=w, in_=ap)
                return w

            Wvo_sb = load_w(Wvo8, "Wvo")
            Wf1_sb = load_w(Wf18, "Wf1")
            Wf2_sb = load_w(Wf28, "Wf2")

            if not lean:
                g1_bc = cn.tile([P, D], BF)
                nc.sync.dma_start(
                    out=g1_bc, in_=gb[0:1, :].to_broadcast([P, D]))
                g2_bc = cn.tile([P, D], BF)
                nc.sync.dma_start(
                    out=g2_bc, in_=gb[1:2, :].to_broadcast([P, D]))
                b2_bc = cn.tile([P, D], BF)
                nc.sync.dma_start(
                    out=b2_bc, in_=gb[2:3, :].to_broadcast([P, D]))
                b1_bc = cn.tile([P, D], BF)
                nc.sync.dma_start(
                    out=b1_bc, in_=gb[3:4, :].to_broadcast([P, D]))
                fb_sb = cn.tile([2, 2, D], BF)
                nc.sync.dma_start(out=fb_sb, in_=fb)
                ones2 = cn.tile([2, P], BF)
                nc.vector.memset(ones2[0:1, :], 1.0)
                nc.vector.memset(ones2[1:2, :], 0.0)

            def ln_coeffs(src, eps_t, scale, tag):
                """bn stats + rstd/bias for ACT apply: (src - m) * q."""
                st = stat.tile([P, NH, 6], F32, tag=f"st{tag}")
                for h in range(NH):
                    nc.vector.bn_stats(
                        out=st[:, h, :], in_=src[:, h * NF:(h + 1) * NF])
                mv = stat.tile([P, 2], F32, tag=f"mv{tag}")
                nc.vector.bn_aggr(out=mv, in_=st)
                s = stat.tile([P, 1], F32, tag=f"s{tag}")
                nc.scalar.activation(
                    out=s, in_=mv[:, 1:2], func=AF.Sqrt,
                    bias=eps_t, scale=scale,
                )
                q = stat.tile([P, 1], F32, tag=f"q{tag}")
                nc.vector.reciprocal(out=q, in_=s)
                mb = stat.tile([P, 1], F32, tag=f"mb{tag}")
                nc.vector.tensor_scalar(
                    out=mb, in0=mv[:, 0:1], scalar1=q, scalar2=-1.0,
                    op0=ALU.mult, op1=ALU.mult,
                )
                return q, mb

            for j in range(NT):
                jsl = slice(j * P, (j + 1) * P)
                x_t = xpool.tile([P, D], BF, tag="x", name="x")
                nc.sync.dma_start(out=x_t, in_=x256[jsl, :])
                xp_t = xppool.tile([P, D], BF, tag="xp", name="xp")
                nc.sync.dma_start(out=xp_t, in_=xp256[jsl, :])
                if not lean:
                    xb_t = xpool.tile([P, D], BF, tag="xb", name="xb")
                    nc.sync.dma_start(out=xb_t, in_=xb256[jsl, :])

                # ---- prefix-mean, transposed: pt[d, t] = 4096 * P^T ----
                pt8 = p8pool.tile([P, KC, P], F8, tag="p8", name="PT8")
                for g in range(2):
                    pts = pfx.tile([P, 4, P], F32, tag="pf", name="pts")
                    for k4 in range(4):
                        kc = 4 * g + k4
                        nc.tensor.matmul(
                            pts[:, k4, :],
                            lhsT=xp_t[:, kc * P:(kc + 1) * P],
                            rhs=ut_sb[:, j, :],
                            start=True, stop=True,
                        )
                    nc.scalar.activation(
                        out=pt8[:, 4 * g:4 * g + 4, :], in_=pts,
                        func=AF.Copy, scale=1.0 / 256.0,
                    )

                # ---- AO = 16P @ 64Wvo = 1024*AO;  r1 = 256*(AO + x) ----
                r1 = r1pool.tile([P, D], BF, tag="r1", name="r1")
                for n in range(NH):
                    nsl = slice(n * NF, (n + 1) * NF)
                    ao = pao.tile([P, NF], F32, tag="ao")
                    for g in range(4):
                        gsl = slice(2 * g, 2 * g + 2)
                        nc.tensor.matmul(
                            ao, lhsT=pt8[:, gsl, :],
                            rhs=Wvo_sb[:, gsl, nsl],
                            start=(g == 0), stop=(g == 3), perf_mode=DR,
                        )
                    nc.vector.scalar_tensor_tensor(
                        out=r1[:, nsl], in0=ao, scalar=0.25,
                        in1=(x_t if lean else xb_t)[:, nsl],
                        op0=ALU.mult, op1=ALU.add,
                    )

                # ---- LN1: n18b = 256*N1 ; n1x = 256*(N1 + x) ----
                q1, mb1 = ln_coeffs(r1, eps1, 1.0 / 65536.0, "1")
                n18 = n8pool.tile([P, D], BF, tag="n8", name="N18")
                nc.scalar.activation(
                    out=n18, in_=r1, func=AF.Identity, bias=mb1, scale=q1)
                if not lean:
                    nc.vector.tensor_mul(out=n18, in0=n18, in1=g1_bc)
                n1x = nxpool.tile([P, D], BF, tag="nx", name="N1x")
                nc.gpsimd.tensor_add(out=n1x, in0=n18, in1=x_t)
                if not lean:
                    nc.gpsimd.tensor_add(out=n1x, in0=n1x, in1=b1_bc)

                # ---- FFN1: psum = 16N1 @ 64Wf1; h8 = 4*relu(N1@Wf1) ----
                t1 = ptp.tile([P, KC, P], BF, tag="tp")
                for kc in range(KC):
                    nc.tensor.transpose(
                        t1[:, kc, :], n18[:, kc * P:(kc + 1) * P], identb)
                n1t = tpool.tile([P, KC, P], F8, tag="t", name="N1T")
                nc.scalar.activation(
                    out=n1t, in_=t1, func=AF.Copy, scale=1.0 / 16.0)

                h8 = h8pool.tile([P, D], BF, tag="h8", name="H8")
                for n in range(NH):
                    nsl = slice(n * NF, (n + 1) * NF)
                    f1 = pf.tile([P, NF], F32, tag="f")
                    for g in range(4):
                        gsl = slice(2 * g, 2 * g + 2)
                        nc.tensor.matmul(
                            f1, lhsT=n1t[:, gsl, :],
                            rhs=Wf1_sb[:, gsl, nsl],
                            start=(g == 0), stop=(g == 3 and lean),
                            perf_mode=DR,
                        )
                    if not lean:
                        nc.tensor.matmul(
                            f1, lhsT=ones2, rhs=fb_sb[:, 0, nsl],
                            start=False, stop=True,
                        )
                    nc.scalar.activation(
                        out=h8[:, nsl], in_=f1, func=AF.Relu,
                        scale=1.0 / 256.0,
                    )

                # ---- FFN2: z = 256*(ff + N1 + x) ----
                t2 = ptp.tile([P, KC, P], BF, tag="tp")
                for kc in range(KC):
                    nc.tensor.transpose(
                        t2[:, kc, :], h8[:, kc * P:(kc + 1) * P], identb)
                ht = tpool.tile([P, KC, P], F8, tag="t", name="HT")
                nc.vector.tensor_copy(out=ht, in_=t2)

                z = zpool.tile([P, D], BF, tag="z", name="z")
                for n in range(NH):
                    nsl = slice(n * NF, (n + 1) * NF)
                    f2 = pf.tile([P, NF], F32, tag="f")
                    for g in range(4):
                        gsl = slice(2 * g, 2 * g + 2)
                        nc.tensor.matmul(
                            f2, lhsT=ht[:, gsl, :],
                            rhs=Wf2_sb[:, gsl, nsl],
                            start=(g == 0), stop=(g == 3 and lean),
                            perf_mode=DR,
                        )
                    if not lean:
                        nc.tensor.matmul(
                            f2, lhsT=ones2, rhs=fb_sb[:, 1, nsl],
                            start=False, stop=True,
                        )
                    nc.vector.tensor_add(
                        out=z[:, nsl], in0=f2, in1=n1x[:, nsl])

                # ---- LN2 -> out ----
                q2, mb2 = ln_coeffs(z, eps2, 1.0, "2")
                o = opool.tile([P, D], BF, tag="o", name="o")
                if lean:
                    nc.scalar.activation(
                        out=o, in_=z, func=AF.Identity, bias=mb2, scale=q2)
                else:
                    op = opool.tile([P, D], BF, tag="op", name="op")
                    nc.scalar.activation(
                        out=op, in_=z, func=AF.Identity, bias=mb2, scale=q2)
                    nc.vector.tensor_mul(out=op, in0=op, in1=g2_bc)
                    nc.vector.tensor_add(out=o, in0=op, in1=b2_bc)
                nc.sync.dma_start(out=out_bf[jsl, :], in_=o)

    nc.compile()
    return nc


_CACHE = {}


def _get_nc(lean=True):
    key = "lean" if lean else "general"
    if key not in _CACHE:
        _CACHE[key] = _build(lean=lean)
    return _CACHE[key]


BF_NP = ml_dtypes.bfloat16
F8_NP = ml_dtypes.float8_e4m3


def _in_maps(x, Wv, Wo, bo, g1, b1, Wf1, bf1, Wf2, bf2, g2, b2, lean=True):
    x = np.asarray(x, dtype=np.float32)
    Wv_all = np.asarray(Wv, np.float32).transpose(1, 0, 2).reshape(D, D)
    Wvo = Wv_all @ np.asarray(Wo, np.float32)

    def wprep(w):
        # [D, D] -> [P, KC, D] so each partition's SBUF row is contiguous
        w8 = np.asarray(64.0 * w, F8_NP)
        return np.ascontiguousarray(
            w8.reshape(KC, P, D).transpose(1, 0, 2))

    base = {
        "Wvo8": wprep(Wvo),
        "Wf18": wprep(np.asarray(Wf1, np.float32)),
        "Wf28": wprep(np.asarray(Wf2, np.float32)),
    }
    if not lean:
        base["gb"] = np.asarray(np.stack([
            np.asarray(g1, np.float32),
            np.asarray(g2, np.float32),
            np.asarray(b2, np.float32),
            256.0 * np.asarray(b1, np.float32),
        ]), BF_NP)
        base["fb"] = np.asarray(np.stack([
            np.stack([1024.0 * (np.asarray(b1, np.float32) @
                                np.asarray(Wf1, np.float32) +
                                np.asarray(bf1, np.float32)),
                      np.zeros(D, np.float32)]),
            np.stack([256.0 * np.asarray(bf2, np.float32),
                      np.zeros(D, np.float32)]),
        ]), BF_NP)

    # per-half triangular tables: ut[u, j, t] = 16/cnt_t for u <= t
    uts = []
    for half in range(2):
        t0 = half * TH
        ut = np.zeros((P, NT, P), np.float32)
        for j in range(NT):
            cnt = (t0 + j * P + np.arange(P) + 1.0).astype(np.float32)
            ut[:, j, :] = np.triu(np.ones((P, P), np.float32)) * (16.0 / cnt)
        uts.append(np.asarray(ut, BF_NP))

    in_maps = []
    for c in range(8):
        b, half = divmod(c, 2)
        t0 = half * TH
        m = dict(base)
        xh = x[b, t0:t0 + TH]
        m["x256"] = np.asarray(256.0 * xh, BF_NP)
        if not lean:
            m["xb256"] = np.asarray(
                256.0 * (xh + np.asarray(bo, np.float32)), BF_NP)
        # xp = x with the cumulative column-sum carry folded into each
        # tile's first row (the inclusive triangular matrix propagates
        # row 0 to every row of the tile)
        ts_sums = x[b].reshape(2 * NT, P, D).sum(axis=1, dtype=np.float64)
        starts = np.zeros((2 * NT, D), np.float64)
        starts[1:] = np.cumsum(ts_sums[:-1], axis=0)
        xp = 256.0 * xh.astype(np.float64)
        for j in range(NT):
            xp[j * P] += 256.0 * starts[half * NT + j]
        m["xp256"] = np.asarray(xp.astype(np.float32), BF_NP)
        m["ut16"] = uts[half]
        in_maps.append(m)
    return in_maps


def _assemble(results):
    out = np.empty((B, T, D), np.float32)
    for c in range(8):
        b, half = divmod(c, 2)
        out[b, half * TH:(half + 1) * TH] = (
            results[c]["out_bf"].astype(np.float32))
    return out


def kernel(x, Wk, Wv, Wo, bo, g1, b1, Wf1, bf1, Wf2, bf2, g2, b2):
    lean = bool(
        not np.any(np.asarray(bo)) and not np.any(np.asarray(bf1))
        and not np.any(np.asarray(bf2)) and not np.any(np.asarray(b1))
        and not np.any(np.asarray(b2))
        and np.all(np.asarray(g1) == 1.0) and np.all(np.asarray(g2) == 1.0)
    )
    in_maps = _in_maps(
        x, Wv, Wo, bo, g1, b1, Wf1, bf1, Wf2, bf2, g2, b2, lean=lean)
    res = run_bass_kernel_spmd(_get_nc(lean), in_maps, list(range(8))).results
    return _assemble(res)
